# revision 19
# baseline (speedup 1.0000x reference)
"""Trainium2 Bass kernel for CRF Viterbi decode (nn_CRFLayer).

Problem: emissions [512, 1024, 48] f32, mask [512,1024] (unused by reference),
transitions [48,48], start/end_transitions [48]. Output: best_paths [512, 1024]
int32 (Viterbi argmax decode, jax reference semantics: first-occurrence argmax).

Strategy (8 NeuronCores, pure data parallel over batch, 64 seqs/core):

Forward (per core): scores s_t[b, j] kept in SBUF, batch on 64 partitions,
current-tag axis j halved across two 64-partition groups (128 lanes total):
  cand[b,(j,i)] = s_{t-1}[b,i] + T[i,j]   (DVE tensor_tensor, bcast AP)
  pre[b,j]     = max_i cand               (DVE grouped tensor_reduce)
  s_t[b,j]     = pre + em_t[b,j]          (DVE tensor_add)
Optionally the last POOL_J j-columns of each half's cand are computed by the
GPSIMD (Pool) engine in parallel with the DVE's columns; the DVE then runs
two grouped reduces (its own columns first, Pool's after they land).
Scores streamed to DRAM scratch in blocks (for exact backtrace recompute).

Backtrace: tag_t = argmax_i(s_t[b,i] + T[i, tag_{t+1}]) recomputed per step.
The 64 sequences are split into BT_CHAINS independent sub-chains that are
interleaved, so the per-step serial chain (DVE -> PE -> Act -> PE -> DVE)
pipelines across chains and the engines stay busy. Per chain and step:
  c48 = hist_t + tcol (TTR, fused max accum m1)
  tag = max_index(m1 bcast, c48)           (first-occurrence, exact)
  tagf = cast tag (Act)
  oh  = is_eq(iota48, tagf)                (one-hot from the *integer* tag --
                                            tie-safe vs comparing c48 to m1)
  ohT = PE matmul (transpose via identity block)
  ohT_sb = Act copy PSUM->SBUF
  tcol = PE matmul(ohT_sb, T^T)            (per-batch column gather)
All arithmetic is bit-exact vs the jax reference (single fp32 adds, exact max,
first-occurrence argmax), so integer paths match exactly.
"""

import os
import sys
from contextlib import ExitStack

import numpy as np

sys.path.insert(0, "/opt/trn_rl_repo")

import concourse.bass as bass  # noqa: E402
import concourse.tile as tile  # noqa: E402
from concourse import bacc, mybir  # noqa: E402

F32 = mybir.dt.float32
U16 = mybir.dt.uint16
I32 = mybir.dt.int32

NUM_TAGS = 48
BATCH = 512
SEQ_LEN = 1024
N_CORES = 8
B_LOC = BATCH // N_CORES  # 64 sequences per core

NEG_INF = float(np.float32(-1e30))
USE_TTR = False
POOL_J = 0


def build_nc(
    S: int = SEQ_LEN,
    TB: int = 128,
    B: int = B_LOC,
    T: int = NUM_TAGS,
    fwd_only: bool = False,
    reps: int = 1,
    pool_j: int = 0,
    hist_out: bool = False,
):
    """Build the per-core Bass program (same program on all cores, SPMD).

    pool_j: j-columns per group-half of forward cand computed on GPSIMD.
    """
    assert S % TB == 0
    nblk = S // TB
    H = T // 2  # 24, j-half width
    IDENT32 = list(range(32))
    JD = H - pool_j  # DVE j-columns per half

    nc = bacc.Bacc("TRN2", target_bir_lowering=False, debug=False, num_devices=N_CORES)

    em_d = nc.dram_tensor("emissions", [B, S, T], F32, kind="ExternalInput")
    trans_d = nc.dram_tensor("transitions", [T, T], F32, kind="ExternalInput")
    start_d = nc.dram_tensor("start_transitions", [T], F32, kind="ExternalInput")
    end_d = nc.dram_tensor("end_transitions", [T], F32, kind="ExternalInput")
    paths_d = nc.dram_tensor("paths", [B, S], I32, kind="ExternalOutput")
    hist_d = nc.dram_tensor(
        "hist", [B, S, T], F32, kind="ExternalOutput" if hist_out else "Internal"
    )  # scratch: forward scores

    with tile.TileContext(nc) as tc, ExitStack() as ctx:
        const = ctx.enter_context(tc.tile_pool(name="const", bufs=1))
        emp = ctx.enter_context(tc.tile_pool(name="emp", bufs=2))
        hip = ctx.enter_context(tc.tile_pool(name="hip", bufs=2))
        wrk = ctx.enter_context(tc.tile_pool(name="wrk", bufs=2))
        psum = ctx.enter_context(tc.tile_pool(name="psum", bufs=2, space="PSUM"))

        # ---- constants -------------------------------------------------
        # Trep[b, j, i] = T[i, j]  (j-major candidate layout)
        t_ap = trans_d.ap()  # [i, j]
        tt_flat = const.tile([1, T * T], F32)
        nc.sync.dma_start(
            tt_flat[:].rearrange("p (j i) -> p j i", j=T), t_ap.transpose([1, 0]).unsqueeze(0)
        )
        # T_T[j, i] = T[i, j] on 48 partitions (rhs of the gather matmul)
        t_t = const.tile([T, T], F32)
        nc.sync.dma_start(t_t[:], t_ap.transpose([1, 0]))
        # end broadcast over batch partitions
        end_b = const.tile([B, T], F32)
        nc.sync.dma_start(end_b[:], end_d.ap().unsqueeze(0).broadcast_to([B, T]))

        # diag01[b, b'] = 1.0 iff b == b' (identity, rhs of the tag-transpose mm)
        diag_i = const.tile([B, B], I32)
        nc.gpsimd.iota(diag_i[:], pattern=[[1, B]], base=0, channel_multiplier=-1)
        diag01 = const.tile([B, B], F32)
        nc.vector.tensor_scalar(diag01[:], diag_i[:], 0, None, op0=mybir.AluOpType.is_equal)
        # iota_p[j, b] = j  (partition index, f32, on 48 partitions)
        iota_p_i = const.tile([T, B], I32)
        nc.gpsimd.iota(iota_p_i[:], pattern=[[0, B]], base=0, channel_multiplier=1)
        iota_p = const.tile([T, B], F32)
        nc.vector.tensor_copy(iota_p[:], iota_p_i[:])

        # path8[b, t, 0:8]: max_index writes full 8-wide rows; col 0 is the tag
        path8 = const.tile([B, S, 8], U16)

        # Trep2[g*64+b, j_lo, i] = T[i, 24g + j_lo]
        # NB: partition_broadcast ignores input free offsets on HW -- each
        # source must sit at offset 0 of its own tile.
        tt_hi = const.tile([1, H * T], F32)
        nc.sync.dma_start(
            tt_hi[:].rearrange("p (j i) -> p j i", j=H),
            t_ap.transpose([1, 0])[H:T].unsqueeze(0),
        )
        # partition_broadcast also cannot write a partition-offset output
        # range on HW -- broadcast at base 0, then stream_shuffle up.
        trep2 = const.tile([2 * B, H, T], F32)
        nc.gpsimd.partition_broadcast(
            trep2[0:B].rearrange("p j i -> p (j i)"), tt_flat[:, 0 : H * T]
        )
        tmp_hi = const.tile([B, H, T], F32)
        nc.gpsimd.partition_broadcast(tmp_hi[:].rearrange("p j i -> p (j i)"), tt_hi[:])
        nc.vector.stream_shuffle(
            trep2[B : 2 * B].rearrange("p j i -> p (j i)"),
            tmp_hi[:].rearrange("p j i -> p (j i)"),
            mask=IDENT32,
        )
        # start2[g*64+b, j_lo] = start[24g + j_lo]
        start2 = const.tile([2 * B, H], F32)
        nc.sync.dma_start(start2[0:B], start_d.ap()[0:H].unsqueeze(0).broadcast_to([B, H]))
        nc.sync.dma_start(
            start2[B : 2 * B], start_d.ap()[H:T].unsqueeze(0).broadcast_to([B, H])
        )

        def assemble_full(s_half):
            """[128, H] half-scores -> [128, T] replicated full scores."""
            sf = wrk.tile([2 * B, T], F32, tag="sfull")
            nc.vector.tensor_copy(sf[0:B, 0:H], s_half[0:B])
            nc.vector.tensor_copy(sf[B : 2 * B, H:T], s_half[B : 2 * B])
            nc.vector.stream_shuffle(sf[0:B, H:T], s_half[B : 2 * B], mask=IDENT32)
            nc.vector.stream_shuffle(sf[B : 2 * B, 0:H], s_half[0:B], mask=IDENT32)
            return sf

        for _rep in range(reps):
            # ---- forward ---------------------------------------------------
            s_full = None
            for blk in range(nblk):
                em_t = emp.tile([2 * B, TB, H], F32, tag="em")
                nc.sync.dma_start(em_t[0:B], em_d.ap()[:, blk * TB : (blk + 1) * TB, 0:H])
                nc.sync.dma_start(
                    em_t[B : 2 * B], em_d.ap()[:, blk * TB : (blk + 1) * TB, H:T]
                )
                hist_t = hip.tile([2 * B, TB, H], F32, tag="hist")

                for off in range(TB):
                    t = blk * TB + off
                    if t == 0:
                        nc.vector.tensor_add(hist_t[:, 0, :], start2[:], em_t[:, 0, :])
                    else:
                        cand = wrk.tile([2 * B, H, T], F32, tag="cand")
                        sb = s_full[:].unsqueeze(1)
                        if pool_j > 0:
                            nc.gpsimd.tensor_tensor(
                                cand[:, JD:H, :],
                                sb.broadcast_to([2 * B, pool_j, T]),
                                trep2[:, JD:H, :],
                                op=mybir.AluOpType.add,
                            )
                            nc.vector.tensor_tensor(
                                cand[:, 0:JD, :],
                                sb.broadcast_to([2 * B, JD, T]),
                                trep2[:, 0:JD, :],
                                op=mybir.AluOpType.add,
                            )
                            pre = wrk.tile([2 * B, H], F32, tag="pre")
                            nc.vector.tensor_reduce(
                                pre[:, 0:JD], cand[:, 0:JD, :],
                                axis=mybir.AxisListType.X, op=mybir.AluOpType.max,
                            )
                            nc.vector.tensor_reduce(
                                pre[:, JD:H], cand[:, JD:H, :],
                                axis=mybir.AxisListType.X, op=mybir.AluOpType.max,
                            )
                        else:
                            nc.vector.tensor_tensor(
                                cand[:],
                                sb.broadcast_to([2 * B, H, T]),
                                trep2[:],
                                op=mybir.AluOpType.add,
                            )
                            pre = wrk.tile([2 * B, H], F32, tag="pre")
                            nc.vector.tensor_reduce(
                                pre[:], cand[:], axis=mybir.AxisListType.X,
                                op=mybir.AluOpType.max,
                            )
                        nc.vector.tensor_add(hist_t[:, off, :], pre[:], em_t[:, off, :])
                    s_full = assemble_full(hist_t[:, off, :])

                nc.sync.dma_start(
                    hist_d.ap()[:, blk * TB : (blk + 1) * TB, 0:H], hist_t[0:B]
                )
                nc.sync.dma_start(
                    hist_d.ap()[:, blk * TB : (blk + 1) * TB, H:T], hist_t[B : 2 * B]
                )

            # ---- final argmax ----------------------------------------------
            fin = const.tile([B, T], F32)
            nc.vector.tensor_add(fin[:], s_full[0:B, :], end_b[:])
            m8f = const.tile([B, 8], F32)
            nc.vector.max(m8f[:], fin[:])
            nc.vector.max_index(path8[:, S - 1, :], m8f[:], fin[:])

            # ---- backtrace -------------------------------------------------
            # tag_t = argmax_i(s_t[b,i] + T[i, tag_{t+1}]), recomputed exactly.
            # Chain per step: cast tag -> PE transpose (bcast lhsT) -> is_eq
            # one-hot (PSUM->SBUF) -> PE column gather -> TTR (add + fused max)
            # -> max_index (first occurrence; tie-safe via integer tags).
            for rblk in (range(nblk - 1, -1, -1) if not fwd_only else []):
                hr = hip.tile([B, TB, T], F32, tag="histr")
                nc.sync.dma_start(hr[:], hist_d.ap()[:, rblk * TB : (rblk + 1) * TB, :])
                for off in range(TB - 1, -1, -1):
                    t = rblk * TB + off
                    if t == S - 1:
                        continue
                    wrep = wrk.tile([B, T], F32, tag="wrep")
                    nc.vector.tensor_copy(
                        wrep[:], path8[:, t + 1, 0:1].broadcast_to([B, T])
                    )
                    tagb = psum.tile([T, B], F32, tag="tagb")
                    nc.tensor.matmul(tagb[:], wrep[:], diag01[:])
                    oht = wrk.tile([T, B], F32, tag="oht")
                    nc.vector.tensor_tensor(
                        oht[:], iota_p[:], tagb[:], op=mybir.AluOpType.is_equal
                    )
                    tcol = psum.tile([B, T], F32, tag="tcol")
                    nc.tensor.matmul(tcol[:], oht[:], t_t[:])
                    c48 = wrk.tile([B, T], F32, tag="c48")
                    if USE_TTR:
                        m1 = wrk.tile([B, 1], F32, tag="m1")
                        nc.vector.tensor_tensor_reduce(
                            c48[:],
                            hr[:, off, :],
                            tcol[:],
                            1.0,
                            NEG_INF,
                            op0=mybir.AluOpType.add,
                            op1=mybir.AluOpType.max,
                            accum_out=m1[:],
                        )
                        nc.vector.max_index(
                            path8[:, t, :],
                            m1[:, 0:1].broadcast_to([B, 8]),
                            c48[:],
                        )
                    else:
                        nc.vector.tensor_add(c48[:], hr[:, off, :], tcol[:])
                        m8 = wrk.tile([B, 8], F32, tag="m8")
                        nc.vector.max(m8[:], c48[:])
                        nc.vector.max_index(path8[:, t, :], m8[:], c48[:])

            # ---- emit paths -------------------------------------------------
            paths_i = const.tile([B, S], I32)
            nc.vector.tensor_copy(paths_i[:], path8[:, :, 0])
            nc.sync.dma_start(paths_d.ap()[:], paths_i[:])

    nc.compile()
    return nc


def kernel(emissions, mask, transitions, start_transitions, end_transitions):
    """Full-input entry point: shards batch over 8 cores, runs SPMD, gathers."""
    from concourse.bass_utils import run_bass_kernel_spmd

    emissions = np.ascontiguousarray(np.asarray(emissions), dtype=np.float32)
    transitions = np.ascontiguousarray(np.asarray(transitions), dtype=np.float32)
    start_transitions = np.ascontiguousarray(np.asarray(start_transitions), dtype=np.float32)
    end_transitions = np.ascontiguousarray(np.asarray(end_transitions), dtype=np.float32)

    nc = build_nc(pool_j=POOL_J)
    in_maps = []
    for c in range(N_CORES):
        sl = emissions[c * B_LOC : (c + 1) * B_LOC]
        in_maps.append(
            {
                "emissions": sl,
                "transitions": transitions,
                "start_transitions": start_transitions,
                "end_transitions": end_transitions,
            }
        )
    res = run_bass_kernel_spmd(nc, in_maps, list(range(N_CORES)))
    out = np.concatenate([r["paths"] for r in res.results], axis=0)
    return out.astype(np.int32)


# revision 21
# speedup vs baseline: 1.3217x; 1.3217x over previous
"""Trainium2 Bass kernel for CRF Viterbi decode (nn_CRFLayer).

Problem: emissions [512, 1024, 48] f32, mask [512,1024] (unused by reference),
transitions [48,48], start/end_transitions [48]. Output: best_paths [512, 1024]
int32 (Viterbi argmax decode, jax reference semantics: first-occurrence argmax).

Strategy (8 NeuronCores, pure data parallel over batch, 64 seqs/core):

Forward (per core): scores s_t[b, j] kept in SBUF, batch on 64 partitions,
current-tag axis j halved across two 64-partition groups (128 lanes total):
  cand[b,(j,i)] = s_{t-1}[b,i] + T[i,j]   (DVE tensor_tensor, bcast AP)
  pre[b,j]     = max_i cand               (DVE grouped tensor_reduce)
  s_t[b,j]     = pre + em_t[b,j]          (DVE tensor_add)
Optionally the last POOL_J j-columns of each half's cand are computed by the
GPSIMD (Pool) engine in parallel with the DVE's columns; the DVE then runs
two grouped reduces (its own columns first, Pool's after they land).
Scores streamed to DRAM scratch in blocks (for exact backtrace recompute).

Backtrace: tag_t = argmax_i(s_t[b,i] + T[i, tag_{t+1}]) recomputed per step.
The 64 sequences are split into BT_CHAINS independent sub-chains that are
interleaved, so the per-step serial chain (DVE -> PE -> Act -> PE -> DVE)
pipelines across chains and the engines stay busy. Per chain and step:
  c48 = hist_t + tcol (TTR, fused max accum m1)
  tag = max_index(m1 bcast, c48)           (first-occurrence, exact)
  tagf = cast tag (Act)
  oh  = is_eq(iota48, tagf)                (one-hot from the *integer* tag --
                                            tie-safe vs comparing c48 to m1)
  ohT = PE matmul (transpose via identity block)
  ohT_sb = Act copy PSUM->SBUF
  tcol = PE matmul(ohT_sb, T^T)            (per-batch column gather)
All arithmetic is bit-exact vs the jax reference (single fp32 adds, exact max,
first-occurrence argmax), so integer paths match exactly.
"""

import os
import sys
from contextlib import ExitStack

import numpy as np

sys.path.insert(0, "/opt/trn_rl_repo")

import concourse.bass as bass  # noqa: E402
import concourse.tile as tile  # noqa: E402
from concourse import bacc, mybir  # noqa: E402

F32 = mybir.dt.float32
U16 = mybir.dt.uint16
I32 = mybir.dt.int32

NUM_TAGS = 48
BATCH = 512
SEQ_LEN = 1024
N_CORES = 8
B_LOC = BATCH // N_CORES  # 64 sequences per core

NEG_INF = float(np.float32(-1e30))
USE_TTR = False
POOL_J = 0
SPLIT_FWD = False


def build_nc(
    S: int = SEQ_LEN,
    TB: int = 128,
    B: int = B_LOC,
    T: int = NUM_TAGS,
    fwd_only: bool = False,
    reps: int = 1,
    pool_j: int = 0,
    hist_out: bool = False,
):
    """Build the per-core Bass program (same program on all cores, SPMD).

    pool_j: j-columns per group-half of forward cand computed on GPSIMD.
    """
    assert S % TB == 0
    nblk = S // TB
    H = T // 2  # 24, j-half width
    IDENT32 = list(range(32))
    JD = H - pool_j  # DVE j-columns per half

    nc = bacc.Bacc("TRN2", target_bir_lowering=False, debug=False, num_devices=N_CORES)

    em_d = nc.dram_tensor("emissions", [B, S, T], F32, kind="ExternalInput")
    trans_d = nc.dram_tensor("transitions", [T, T], F32, kind="ExternalInput")
    start_d = nc.dram_tensor("start_transitions", [T], F32, kind="ExternalInput")
    end_d = nc.dram_tensor("end_transitions", [T], F32, kind="ExternalInput")
    paths_d = nc.dram_tensor("paths", [B, S], I32, kind="ExternalOutput")
    hist_d = nc.dram_tensor(
        "hist", [B, S, T], F32, kind="ExternalOutput" if hist_out else "Internal"
    )  # scratch: forward scores

    with tile.TileContext(nc) as tc, ExitStack() as ctx:
        const = ctx.enter_context(tc.tile_pool(name="const", bufs=1))
        emp = ctx.enter_context(tc.tile_pool(name="emp", bufs=2))
        hip = ctx.enter_context(tc.tile_pool(name="hip", bufs=2))
        wrk = ctx.enter_context(tc.tile_pool(name="wrk", bufs=2))
        psum = ctx.enter_context(tc.tile_pool(name="psum", bufs=2, space="PSUM"))

        # ---- constants -------------------------------------------------
        # Trep[b, j, i] = T[i, j]  (j-major candidate layout)
        t_ap = trans_d.ap()  # [i, j]
        tt_flat = const.tile([1, T * T], F32)
        nc.sync.dma_start(
            tt_flat[:].rearrange("p (j i) -> p j i", j=T), t_ap.transpose([1, 0]).unsqueeze(0)
        )
        # T_T[j, i] = T[i, j] on 48 partitions (rhs of the gather matmul)
        t_t = const.tile([T, T], F32)
        nc.sync.dma_start(t_t[:], t_ap.transpose([1, 0]))
        # end broadcast over batch partitions
        end_b = const.tile([B, T], F32)
        nc.sync.dma_start(end_b[:], end_d.ap().unsqueeze(0).broadcast_to([B, T]))

        # diag01[b, b'] = 1.0 iff b == b' (identity, rhs of the tag-transpose mm)
        diag_i = const.tile([B, B], I32)
        nc.gpsimd.iota(diag_i[:], pattern=[[1, B]], base=0, channel_multiplier=-1)
        diag01 = const.tile([B, B], F32)
        nc.vector.tensor_scalar(diag01[:], diag_i[:], 0, None, op0=mybir.AluOpType.is_equal)
        # iota_p[j, b] = j  (partition index, f32, on 48 partitions)
        iota_p_i = const.tile([T, B], I32)
        nc.gpsimd.iota(iota_p_i[:], pattern=[[0, B]], base=0, channel_multiplier=1)
        iota_p = const.tile([T, B], F32)
        nc.vector.tensor_copy(iota_p[:], iota_p_i[:])

        # path8[b, t, 0:8]: max_index writes full 8-wide rows; col 0 is the tag
        path8 = const.tile([B, S, 8], U16)

        if not SPLIT_FWD:
            # Trep[b, j, i] = T[i, j] replicated across batch partitions
            trep = const.tile([B, T, T], F32)
            nc.gpsimd.partition_broadcast(
                trep[:].rearrange("p j i -> p (j i)"), tt_flat[:]
            )
            start_b = const.tile([B, T], F32)
            nc.sync.dma_start(
                start_b[:], start_d.ap().unsqueeze(0).broadcast_to([B, T])
            )

        # Trep2[g*64+b, j_lo, i] = T[i, 24g + j_lo]
        # NB: partition_broadcast ignores input free offsets on HW -- each
        # source must sit at offset 0 of its own tile.
        trep2 = start2 = None
        if SPLIT_FWD:
            tt_hi = const.tile([1, H * T], F32)
            nc.sync.dma_start(
                tt_hi[:].rearrange("p (j i) -> p j i", j=H),
                t_ap.transpose([1, 0])[H:T].unsqueeze(0),
            )
            # partition_broadcast also cannot write a partition-offset output
            # range on HW -- broadcast at base 0, then stream_shuffle up.
            trep2 = const.tile([2 * B, H, T], F32)
            nc.gpsimd.partition_broadcast(
                trep2[0:B].rearrange("p j i -> p (j i)"), tt_flat[:, 0 : H * T]
            )
            tmp_hi = const.tile([B, H, T], F32)
            nc.gpsimd.partition_broadcast(tmp_hi[:].rearrange("p j i -> p (j i)"), tt_hi[:])
            nc.vector.stream_shuffle(
                trep2[B : 2 * B].rearrange("p j i -> p (j i)"),
                tmp_hi[:].rearrange("p j i -> p (j i)"),
                mask=IDENT32,
            )
            # start2[g*64+b, j_lo] = start[24g + j_lo]
            start2 = const.tile([2 * B, H], F32)
            nc.sync.dma_start(
                start2[0:B], start_d.ap()[0:H].unsqueeze(0).broadcast_to([B, H])
            )
            nc.sync.dma_start(
                start2[B : 2 * B], start_d.ap()[H:T].unsqueeze(0).broadcast_to([B, H])
            )

        def assemble_full(s_half):
            """[128, H] half-scores -> [128, T] replicated full scores."""
            sf = wrk.tile([2 * B, T], F32, tag="sfull")
            nc.vector.tensor_copy(sf[0:B, 0:H], s_half[0:B])
            nc.vector.tensor_copy(sf[B : 2 * B, H:T], s_half[B : 2 * B])
            nc.vector.stream_shuffle(sf[0:B, H:T], s_half[B : 2 * B], mask=IDENT32)
            nc.vector.stream_shuffle(sf[B : 2 * B, 0:H], s_half[0:B], mask=IDENT32)
            return sf

        for _rep in range(reps):
            # ---- forward ---------------------------------------------------
            s_full = None
            s_last = None
            if SPLIT_FWD:
                for blk in range(nblk):
                    em_t = emp.tile([2 * B, TB, H], F32, tag="em")
                    nc.sync.dma_start(
                        em_t[0:B], em_d.ap()[:, blk * TB : (blk + 1) * TB, 0:H]
                    )
                    nc.sync.dma_start(
                        em_t[B : 2 * B], em_d.ap()[:, blk * TB : (blk + 1) * TB, H:T]
                    )
                    hist_t = hip.tile([2 * B, TB, H], F32, tag="hist")

                    for off in range(TB):
                        t = blk * TB + off
                        if t == 0:
                            nc.vector.tensor_add(
                                hist_t[:, 0, :], start2[:], em_t[:, 0, :]
                            )
                        else:
                            cand = wrk.tile([2 * B, H, T], F32, tag="cand")
                            sb = s_full[:].unsqueeze(1)
                            nc.vector.tensor_tensor(
                                cand[:],
                                sb.broadcast_to([2 * B, H, T]),
                                trep2[:],
                                op=mybir.AluOpType.add,
                            )
                            pre = wrk.tile([2 * B, H], F32, tag="pre")
                            nc.vector.tensor_reduce(
                                pre[:], cand[:], axis=mybir.AxisListType.X,
                                op=mybir.AluOpType.max,
                            )
                            nc.vector.tensor_add(
                                hist_t[:, off, :], pre[:], em_t[:, off, :]
                            )
                        s_full = assemble_full(hist_t[:, off, :])

                    nc.sync.dma_start(
                        hist_d.ap()[:, blk * TB : (blk + 1) * TB, 0:H], hist_t[0:B]
                    )
                    nc.sync.dma_start(
                        hist_d.ap()[:, blk * TB : (blk + 1) * TB, H:T],
                        hist_t[B : 2 * B],
                    )
                s_last = s_full[0:B, :]
            else:
                # unsplit: batch on 64 partitions, full j per step -- only
                # 3 instructions per step (cand, reduce, add-em), no assemble.
                hist_prev = None
                for blk in range(nblk):
                    em_t = emp.tile([B, TB, T], F32, tag="em")
                    nc.sync.dma_start(
                        em_t[:], em_d.ap()[:, blk * TB : (blk + 1) * TB, :]
                    )
                    hist_t = hip.tile([B, TB, T], F32, tag="hist")

                    for off in range(TB):
                        t = blk * TB + off
                        if t == 0:
                            nc.vector.tensor_add(
                                hist_t[:, 0, :], start_b[:], em_t[:, 0, :]
                            )
                        else:
                            s_prev = (
                                hist_t[:, off - 1, :]
                                if off > 0
                                else hist_prev[:, TB - 1, :]
                            )
                            cand = wrk.tile([B, T, T], F32, tag="cand")
                            nc.vector.tensor_tensor(
                                cand[:],
                                s_prev.unsqueeze(1).broadcast_to([B, T, T]),
                                trep[:],
                                op=mybir.AluOpType.add,
                            )
                            pre = wrk.tile([B, T], F32, tag="pre")
                            nc.vector.tensor_reduce(
                                pre[:], cand[:], axis=mybir.AxisListType.X,
                                op=mybir.AluOpType.max,
                            )
                            nc.vector.tensor_add(
                                hist_t[:, off, :], pre[:], em_t[:, off, :]
                            )
                    hist_prev = hist_t

                    nc.sync.dma_start(
                        hist_d.ap()[:, blk * TB : (blk + 1) * TB, :], hist_t[:]
                    )
                s_last = hist_prev[:, TB - 1, :]

            # ---- final argmax ----------------------------------------------
            fin = const.tile([B, T], F32)
            nc.vector.tensor_add(fin[:], s_last, end_b[:])
            m8f = const.tile([B, 8], F32)
            nc.vector.max(m8f[:], fin[:])
            nc.vector.max_index(path8[:, S - 1, :], m8f[:], fin[:])

            # ---- backtrace -------------------------------------------------
            # tag_t = argmax_i(s_t[b,i] + T[i, tag_{t+1}]), recomputed exactly.
            # Chain per step: cast tag -> PE transpose (bcast lhsT) -> is_eq
            # one-hot (PSUM->SBUF) -> PE column gather -> TTR (add + fused max)
            # -> max_index (first occurrence; tie-safe via integer tags).
            for rblk in (range(nblk - 1, -1, -1) if not fwd_only else []):
                hr = hip.tile([B, TB, T], F32, tag="histr")
                nc.sync.dma_start(hr[:], hist_d.ap()[:, rblk * TB : (rblk + 1) * TB, :])
                for off in range(TB - 1, -1, -1):
                    t = rblk * TB + off
                    if t == S - 1:
                        continue
                    wrep = wrk.tile([B, T], F32, tag="wrep")
                    nc.vector.tensor_copy(
                        wrep[:], path8[:, t + 1, 0:1].broadcast_to([B, T])
                    )
                    tagb = psum.tile([T, B], F32, tag="tagb")
                    nc.tensor.matmul(tagb[:], wrep[:], diag01[:])
                    oht = wrk.tile([T, B], F32, tag="oht")
                    nc.vector.tensor_tensor(
                        oht[:], iota_p[:], tagb[:], op=mybir.AluOpType.is_equal
                    )
                    tcol = psum.tile([B, T], F32, tag="tcol")
                    nc.tensor.matmul(tcol[:], oht[:], t_t[:])
                    c48 = wrk.tile([B, T], F32, tag="c48")
                    if USE_TTR:
                        m1 = wrk.tile([B, 1], F32, tag="m1")
                        nc.vector.tensor_tensor_reduce(
                            c48[:],
                            hr[:, off, :],
                            tcol[:],
                            1.0,
                            NEG_INF,
                            op0=mybir.AluOpType.add,
                            op1=mybir.AluOpType.max,
                            accum_out=m1[:],
                        )
                        nc.vector.max_index(
                            path8[:, t, :],
                            m1[:, 0:1].broadcast_to([B, 8]),
                            c48[:],
                        )
                    else:
                        nc.vector.tensor_add(c48[:], hr[:, off, :], tcol[:])
                        m8 = wrk.tile([B, 8], F32, tag="m8")
                        nc.vector.max(m8[:], c48[:])
                        nc.vector.max_index(path8[:, t, :], m8[:], c48[:])

            # ---- emit paths -------------------------------------------------
            paths_i = const.tile([B, S], I32)
            nc.vector.tensor_copy(paths_i[:], path8[:, :, 0])
            nc.sync.dma_start(paths_d.ap()[:], paths_i[:])

    nc.compile()
    return nc


def kernel(emissions, mask, transitions, start_transitions, end_transitions):
    """Full-input entry point: shards batch over 8 cores, runs SPMD, gathers."""
    from concourse.bass_utils import run_bass_kernel_spmd

    emissions = np.ascontiguousarray(np.asarray(emissions), dtype=np.float32)
    transitions = np.ascontiguousarray(np.asarray(transitions), dtype=np.float32)
    start_transitions = np.ascontiguousarray(np.asarray(start_transitions), dtype=np.float32)
    end_transitions = np.ascontiguousarray(np.asarray(end_transitions), dtype=np.float32)

    nc = build_nc(pool_j=POOL_J)
    in_maps = []
    for c in range(N_CORES):
        sl = emissions[c * B_LOC : (c + 1) * B_LOC]
        in_maps.append(
            {
                "emissions": sl,
                "transitions": transitions,
                "start_transitions": start_transitions,
                "end_transitions": end_transitions,
            }
        )
    res = run_bass_kernel_spmd(nc, in_maps, list(range(N_CORES)))
    out = np.concatenate([r["paths"] for r in res.results], axis=0)
    return out.astype(np.int32)


# revision 24
# speedup vs baseline: 1.3932x; 1.0541x over previous
"""Trainium2 Bass kernel for CRF Viterbi decode (nn_CRFLayer).

Problem: emissions [512, 1024, 48] f32, mask [512,1024] (unused by reference),
transitions [48,48], start/end_transitions [48]. Output: best_paths [512, 1024]
int32 (Viterbi argmax decode, jax reference semantics: first-occurrence argmax).

Strategy (8 NeuronCores, pure data parallel over batch, 64 seqs/core):

The execution backend here is ~uniformly instruction-count-bound (~60-90us
per instruction regardless of operand size, measured via in-kernel reps),
so the design minimizes instructions per timestep, not modeled engine cycles.

Forward (per core): scores s_t[b, j] kept in SBUF, batch on 64 partitions,
full tag range per step -- exactly 3 instructions per timestep:
  cand[b,(j,i)] = s_{t-1}[b,i] + T[i,j]   (DVE tensor_tensor, bcast AP)
  pre[b,j]     = max_i cand               (DVE grouped tensor_reduce)
  s_t[b,j]     = pre + em_t[b,j]          (DVE tensor_add, writes hist tile)
(SPLIT_FWD=True restores the older 128-partition j-split variant: lower
modeled DVE time but 7 instructions/step -- slower on this backend.)
Scores streamed to DRAM scratch in blocks (for exact backtrace recompute).

Backtrace: tag_t = argmax_i(s_t[b,i] + T[i, tag_{t+1}]) recomputed per step,
7 instructions per timestep:
  wrep  = tag cast+broadcast u16->f32      (DVE copy)
  tagb  = PE matmul(wrep, identity)        (tag transposed to 48 partitions)
  oht   = is_eq(iota_p, tagb)              (one-hot from the *integer* tag --
                                            tie-safe, PSUM->SBUF bridge)
  tcol  = PE matmul(oht, T^T)              (per-batch column gather)
  c48   = hist_t + tcol; max8; max_index   (first-occurrence argmax)
All arithmetic is bit-exact vs the jax reference (single fp32 adds, exact max,
first-occurrence argmax), so integer paths match exactly.
"""

import os
import sys
from contextlib import ExitStack

import numpy as np

sys.path.insert(0, "/opt/trn_rl_repo")

import concourse.bass as bass  # noqa: E402
import concourse.tile as tile  # noqa: E402
from concourse import bacc, mybir  # noqa: E402

F32 = mybir.dt.float32
U16 = mybir.dt.uint16
I32 = mybir.dt.int32

NUM_TAGS = 48
BATCH = 512
SEQ_LEN = 1024
N_CORES = 8
B_LOC = BATCH // N_CORES  # 64 sequences per core

NEG_INF = float(np.float32(-1e30))
USE_TTR = False
POOL_J = 0
SPLIT_FWD = False
BT_PSUM = False
FWD_POOL = True


def build_nc(
    S: int = SEQ_LEN,
    TB: int = 128,
    B: int = B_LOC,
    T: int = NUM_TAGS,
    fwd_only: bool = False,
    reps: int = 1,
    pool_j: int = 0,
    hist_out: bool = False,
):
    """Build the per-core Bass program (same program on all cores, SPMD).

    pool_j: j-columns per group-half of forward cand computed on GPSIMD.
    """
    assert S % TB == 0
    nblk = S // TB
    H = T // 2  # 24, j-half width
    IDENT32 = list(range(32))
    JD = H - pool_j  # DVE j-columns per half

    nc = bacc.Bacc("TRN2", target_bir_lowering=False, debug=False, num_devices=N_CORES)

    em_d = nc.dram_tensor("emissions", [B, S, T], F32, kind="ExternalInput")
    trans_d = nc.dram_tensor("transitions", [T, T], F32, kind="ExternalInput")
    start_d = nc.dram_tensor("start_transitions", [T], F32, kind="ExternalInput")
    end_d = nc.dram_tensor("end_transitions", [T], F32, kind="ExternalInput")
    paths_d = nc.dram_tensor("paths", [B, S], I32, kind="ExternalOutput")
    hist_d = nc.dram_tensor(
        "hist", [B, S, T], F32, kind="ExternalOutput" if hist_out else "Internal"
    )  # scratch: forward scores

    with tile.TileContext(nc) as tc, ExitStack() as ctx:
        const = ctx.enter_context(tc.tile_pool(name="const", bufs=1))
        emp = ctx.enter_context(tc.tile_pool(name="emp", bufs=2))
        hip = ctx.enter_context(tc.tile_pool(name="hip", bufs=2))
        wrk = ctx.enter_context(tc.tile_pool(name="wrk", bufs=2))
        psum = ctx.enter_context(tc.tile_pool(name="psum", bufs=2, space="PSUM"))

        # ---- constants -------------------------------------------------
        # Trep[b, j, i] = T[i, j]  (j-major candidate layout)
        t_ap = trans_d.ap()  # [i, j]
        tt_flat = const.tile([1, T * T], F32)
        nc.sync.dma_start(
            tt_flat[:].rearrange("p (j i) -> p j i", j=T), t_ap.transpose([1, 0]).unsqueeze(0)
        )
        # T_T[j, i] = T[i, j] on 48 partitions (rhs of the gather matmul)
        t_t = const.tile([T, T], F32)
        nc.sync.dma_start(t_t[:], t_ap.transpose([1, 0]))
        # end broadcast over batch partitions
        end_b = const.tile([B, T], F32)
        nc.sync.dma_start(end_b[:], end_d.ap().unsqueeze(0).broadcast_to([B, T]))

        # diag01[b, b'] = 1.0 iff b == b' (identity, rhs of the tag-transpose mm)
        diag_i = const.tile([B, B], I32)
        nc.gpsimd.iota(diag_i[:], pattern=[[1, B]], base=0, channel_multiplier=-1)
        diag01 = const.tile([B, B], F32)
        nc.vector.tensor_scalar(diag01[:], diag_i[:], 0, None, op0=mybir.AluOpType.is_equal)
        # iota_p[j, b] = j  (partition index, f32, on 48 partitions)
        iota_p_i = const.tile([T, B], I32)
        nc.gpsimd.iota(iota_p_i[:], pattern=[[0, B]], base=0, channel_multiplier=1)
        iota_p = const.tile([T, B], F32)
        nc.vector.tensor_copy(iota_p[:], iota_p_i[:])

        # path8[b, t, 0:8]: max_index writes full 8-wide rows; col 0 is the tag
        path8 = const.tile([B, S, 8], U16)

        if not SPLIT_FWD:
            # Trep[b, j, i] = T[i, j] replicated across batch partitions
            trep = const.tile([B, T, T], F32)
            nc.gpsimd.partition_broadcast(
                trep[:].rearrange("p j i -> p (j i)"), tt_flat[:]
            )
            start_b = const.tile([B, T], F32)
            nc.sync.dma_start(
                start_b[:], start_d.ap().unsqueeze(0).broadcast_to([B, T])
            )

        # Trep2[g*64+b, j_lo, i] = T[i, 24g + j_lo]
        # NB: partition_broadcast ignores input free offsets on HW -- each
        # source must sit at offset 0 of its own tile.
        trep2 = start2 = None
        if SPLIT_FWD:
            tt_hi = const.tile([1, H * T], F32)
            nc.sync.dma_start(
                tt_hi[:].rearrange("p (j i) -> p j i", j=H),
                t_ap.transpose([1, 0])[H:T].unsqueeze(0),
            )
            # partition_broadcast also cannot write a partition-offset output
            # range on HW -- broadcast at base 0, then stream_shuffle up.
            trep2 = const.tile([2 * B, H, T], F32)
            nc.gpsimd.partition_broadcast(
                trep2[0:B].rearrange("p j i -> p (j i)"), tt_flat[:, 0 : H * T]
            )
            tmp_hi = const.tile([B, H, T], F32)
            nc.gpsimd.partition_broadcast(tmp_hi[:].rearrange("p j i -> p (j i)"), tt_hi[:])
            nc.vector.stream_shuffle(
                trep2[B : 2 * B].rearrange("p j i -> p (j i)"),
                tmp_hi[:].rearrange("p j i -> p (j i)"),
                mask=IDENT32,
            )
            # start2[g*64+b, j_lo] = start[24g + j_lo]
            start2 = const.tile([2 * B, H], F32)
            nc.sync.dma_start(
                start2[0:B], start_d.ap()[0:H].unsqueeze(0).broadcast_to([B, H])
            )
            nc.sync.dma_start(
                start2[B : 2 * B], start_d.ap()[H:T].unsqueeze(0).broadcast_to([B, H])
            )

        def assemble_full(s_half):
            """[128, H] half-scores -> [128, T] replicated full scores."""
            sf = wrk.tile([2 * B, T], F32, tag="sfull")
            nc.vector.tensor_copy(sf[0:B, 0:H], s_half[0:B])
            nc.vector.tensor_copy(sf[B : 2 * B, H:T], s_half[B : 2 * B])
            nc.vector.stream_shuffle(sf[0:B, H:T], s_half[B : 2 * B], mask=IDENT32)
            nc.vector.stream_shuffle(sf[B : 2 * B, 0:H], s_half[0:B], mask=IDENT32)
            return sf

        for _rep in range(reps):
            # ---- forward ---------------------------------------------------
            s_full = None
            s_last = None
            if SPLIT_FWD:
                for blk in range(nblk):
                    em_t = emp.tile([2 * B, TB, H], F32, tag="em")
                    nc.sync.dma_start(
                        em_t[0:B], em_d.ap()[:, blk * TB : (blk + 1) * TB, 0:H]
                    )
                    nc.sync.dma_start(
                        em_t[B : 2 * B], em_d.ap()[:, blk * TB : (blk + 1) * TB, H:T]
                    )
                    hist_t = hip.tile([2 * B, TB, H], F32, tag="hist")

                    for off in range(TB):
                        t = blk * TB + off
                        if t == 0:
                            nc.vector.tensor_add(
                                hist_t[:, 0, :], start2[:], em_t[:, 0, :]
                            )
                        else:
                            cand = wrk.tile([2 * B, H, T], F32, tag="cand")
                            sb = s_full[:].unsqueeze(1)
                            nc.vector.tensor_tensor(
                                cand[:],
                                sb.broadcast_to([2 * B, H, T]),
                                trep2[:],
                                op=mybir.AluOpType.add,
                            )
                            pre = wrk.tile([2 * B, H], F32, tag="pre")
                            nc.vector.tensor_reduce(
                                pre[:], cand[:], axis=mybir.AxisListType.X,
                                op=mybir.AluOpType.max,
                            )
                            nc.vector.tensor_add(
                                hist_t[:, off, :], pre[:], em_t[:, off, :]
                            )
                        s_full = assemble_full(hist_t[:, off, :])

                    nc.sync.dma_start(
                        hist_d.ap()[:, blk * TB : (blk + 1) * TB, 0:H], hist_t[0:B]
                    )
                    nc.sync.dma_start(
                        hist_d.ap()[:, blk * TB : (blk + 1) * TB, H:T],
                        hist_t[B : 2 * B],
                    )
                s_last = s_full[0:B, :]
            else:
                # unsplit: batch on 64 partitions, full j per step -- only
                # 3 instructions per step (cand, reduce, add-em), no assemble.
                hist_prev = None
                for blk in range(nblk):
                    em_t = emp.tile([B, TB, T], F32, tag="em")
                    nc.sync.dma_start(
                        em_t[:], em_d.ap()[:, blk * TB : (blk + 1) * TB, :]
                    )
                    hist_t = hip.tile([B, TB, T], F32, tag="hist")

                    for off in range(TB):
                        t = blk * TB + off
                        if t == 0:
                            nc.vector.tensor_add(
                                hist_t[:, 0, :], start_b[:], em_t[:, 0, :]
                            )
                        else:
                            s_prev = (
                                hist_t[:, off - 1, :]
                                if off > 0
                                else hist_prev[:, TB - 1, :]
                            )
                            cand = wrk.tile([B, T, T], F32, tag="cand")
                            eng_tt = nc.gpsimd if FWD_POOL else nc.vector
                            eng_tt.tensor_tensor(
                                cand[:],
                                s_prev.unsqueeze(1).broadcast_to([B, T, T]),
                                trep[:],
                                op=mybir.AluOpType.add,
                            )
                            pre = wrk.tile([B, T], F32, tag="pre")
                            nc.vector.tensor_reduce(
                                pre[:], cand[:], axis=mybir.AxisListType.X,
                                op=mybir.AluOpType.max,
                            )
                            eng_tt.tensor_tensor(
                                hist_t[:, off, :], pre[:], em_t[:, off, :],
                                op=mybir.AluOpType.add,
                            )
                    hist_prev = hist_t

                    nc.sync.dma_start(
                        hist_d.ap()[:, blk * TB : (blk + 1) * TB, :], hist_t[:]
                    )
                s_last = hist_prev[:, TB - 1, :]

            # ---- final argmax ----------------------------------------------
            fin = const.tile([B, T], F32)
            nc.vector.tensor_add(fin[:], s_last, end_b[:])
            m8f = const.tile([B, 8], F32)
            nc.vector.max(m8f[:], fin[:])
            nc.vector.max_index(path8[:, S - 1, :], m8f[:], fin[:])

            # ---- backtrace -------------------------------------------------
            # tag_t = argmax_i(s_t[b,i] + T[i, tag_{t+1}]), recomputed exactly.
            # Chain per step: cast tag -> PE transpose (bcast lhsT) -> is_eq
            # one-hot (PSUM->SBUF) -> PE column gather -> TTR (add + fused max)
            # -> max_index (first occurrence; tie-safe via integer tags).
            TBB = 16 if BT_PSUM else TB
            nbb = S // TBB
            for rblk in (range(nbb - 1, -1, -1) if not fwd_only else []):
                if BT_PSUM:
                    # hist block straight into PSUM; the gather matmul then
                    # accumulates T[:, tag] on top (start=False), so the
                    # separate DVE add disappears.
                    hr = psum.tile([B, TBB, T], F32, tag="histr")
                else:
                    hr = hip.tile([B, TBB, T], F32, tag="histr")
                nc.sync.dma_start(
                    hr[:], hist_d.ap()[:, rblk * TBB : (rblk + 1) * TBB, :]
                )
                for off in range(TBB - 1, -1, -1):
                    t = rblk * TBB + off
                    if t == S - 1:
                        continue
                    wrep = wrk.tile([B, T], F32, tag="wrep")
                    nc.vector.tensor_copy(
                        wrep[:], path8[:, t + 1, 0:1].broadcast_to([B, T])
                    )
                    tagb = psum.tile([T, B], F32, tag="tagb")
                    nc.tensor.matmul(tagb[:], wrep[:], diag01[:])
                    oht = wrk.tile([T, B], F32, tag="oht")
                    nc.vector.tensor_tensor(
                        oht[:], iota_p[:], tagb[:], op=mybir.AluOpType.is_equal
                    )
                    if BT_PSUM:
                        nc.tensor.matmul(
                            hr[:, off, :], oht[:], t_t[:],
                            start=False, stop=True, skip_group_check=True,
                        )
                        m8 = wrk.tile([B, 8], F32, tag="m8")
                        nc.vector.max(m8[:], hr[:, off, :])
                        nc.vector.max_index(path8[:, t, :], m8[:], hr[:, off, :])
                    else:
                        tcol = psum.tile([B, T], F32, tag="tcol")
                        nc.tensor.matmul(tcol[:], oht[:], t_t[:])
                        c48 = wrk.tile([B, T], F32, tag="c48")
                        nc.vector.tensor_add(c48[:], hr[:, off, :], tcol[:])
                        m8 = wrk.tile([B, 8], F32, tag="m8")
                        nc.vector.max(m8[:], c48[:])
                        nc.vector.max_index(path8[:, t, :], m8[:], c48[:])

            # ---- emit paths -------------------------------------------------
            paths_i = const.tile([B, S], I32)
            nc.vector.tensor_copy(paths_i[:], path8[:, :, 0])
            nc.sync.dma_start(paths_d.ap()[:], paths_i[:])

    nc.compile()
    return nc


def kernel(emissions, mask, transitions, start_transitions, end_transitions):
    """Full-input entry point: shards batch over 8 cores, runs SPMD, gathers."""
    from concourse.bass_utils import run_bass_kernel_spmd

    emissions = np.ascontiguousarray(np.asarray(emissions), dtype=np.float32)
    transitions = np.ascontiguousarray(np.asarray(transitions), dtype=np.float32)
    start_transitions = np.ascontiguousarray(np.asarray(start_transitions), dtype=np.float32)
    end_transitions = np.ascontiguousarray(np.asarray(end_transitions), dtype=np.float32)

    nc = build_nc(pool_j=POOL_J)
    in_maps = []
    for c in range(N_CORES):
        sl = emissions[c * B_LOC : (c + 1) * B_LOC]
        in_maps.append(
            {
                "emissions": sl,
                "transitions": transitions,
                "start_transitions": start_transitions,
                "end_transitions": end_transitions,
            }
        )
    res = run_bass_kernel_spmd(nc, in_maps, list(range(N_CORES)))
    out = np.concatenate([r["paths"] for r in res.results], axis=0)
    return out.astype(np.int32)


# revision 26
# speedup vs baseline: 1.7547x; 1.2595x over previous
"""Trainium2 Bass kernel for CRF Viterbi decode (nn_CRFLayer).

Problem: emissions [512, 1024, 48] f32, mask [512,1024] (unused by reference),
transitions [48,48], start/end_transitions [48]. Output: best_paths [512, 1024]
int32 (Viterbi argmax decode, jax reference semantics: first-occurrence argmax).

Strategy (8 NeuronCores, pure data parallel over batch, 64 seqs/core):

The execution backend here is ~uniformly instruction-count-bound (~60-90us
per instruction regardless of operand size, measured via in-kernel reps),
so the design minimizes instructions per timestep, not modeled engine cycles.

Forward (per core): scores s_t[b, j] kept in SBUF, batch on 64 partitions,
full tag range per step -- exactly 3 instructions per timestep:
  cand[b,(j,i)] = s_{t-1}[b,i] + T[i,j]   (DVE tensor_tensor, bcast AP)
  pre[b,j]     = max_i cand               (DVE grouped tensor_reduce)
  s_t[b,j]     = pre + em_t[b,j]          (DVE tensor_add, writes hist tile)
(SPLIT_FWD=True restores the older 128-partition j-split variant: lower
modeled DVE time but 7 instructions/step -- slower on this backend.)
Scores streamed to DRAM scratch in blocks (for exact backtrace recompute).

Backtrace: tag_t = argmax_i(s_t[b,i] + T[i, tag_{t+1}]) recomputed per step,
7 instructions per timestep:
  wrep  = tag cast+broadcast u16->f32      (DVE copy)
  tagb  = PE matmul(wrep, identity)        (tag transposed to 48 partitions)
  oht   = is_eq(iota_p, tagb)              (one-hot from the *integer* tag --
                                            tie-safe, PSUM->SBUF bridge)
  tcol  = PE matmul(oht, T^T)              (per-batch column gather)
  c48   = hist_t + tcol; max8; max_index   (first-occurrence argmax)
All arithmetic is bit-exact vs the jax reference (single fp32 adds, exact max,
first-occurrence argmax), so integer paths match exactly.
"""

import os
import sys
from contextlib import ExitStack

import numpy as np

sys.path.insert(0, "/opt/trn_rl_repo")

import concourse.bass as bass  # noqa: E402
import concourse.tile as tile  # noqa: E402
from concourse import bacc, mybir  # noqa: E402

F32 = mybir.dt.float32
U16 = mybir.dt.uint16
U32 = mybir.dt.uint32
I32 = mybir.dt.int32

NUM_TAGS = 48
BATCH = 512
SEQ_LEN = 1024
N_CORES = 8
B_LOC = BATCH // N_CORES  # 64 sequences per core

NEG_INF = float(np.float32(-1e30))
USE_TTR = False
POOL_J = 0
SPLIT_FWD = False
BT_PSUM = False
FWD_POOL = True


def build_nc(
    S: int = SEQ_LEN,
    TB: int = 128,
    B: int = B_LOC,
    T: int = NUM_TAGS,
    fwd_only: bool = False,
    reps: int = 1,
    pool_j: int = 0,
    hist_out: bool = False,
):
    """Build the per-core Bass program (same program on all cores, SPMD).

    pool_j: j-columns per group-half of forward cand computed on GPSIMD.
    """
    assert S % TB == 0
    nblk = S // TB
    H = T // 2  # 24, j-half width
    IDENT32 = list(range(32))
    JD = H - pool_j  # DVE j-columns per half

    nc = bacc.Bacc("TRN2", target_bir_lowering=False, debug=False, num_devices=N_CORES)

    em_d = nc.dram_tensor("emissions", [B, S, T], F32, kind="ExternalInput")
    trans_d = nc.dram_tensor("transitions", [T, T], F32, kind="ExternalInput")
    start_d = nc.dram_tensor("start_transitions", [T], F32, kind="ExternalInput")
    end_d = nc.dram_tensor("end_transitions", [T], F32, kind="ExternalInput")
    paths_d = nc.dram_tensor("paths", [B, S], I32, kind="ExternalOutput")
    hist_d = nc.dram_tensor(
        "hist", [B, S, T], F32, kind="ExternalOutput" if hist_out else "Internal"
    )  # scratch: forward scores
    ttT_d = nc.dram_tensor("ttT", [T, T], F32, kind="Internal")  # T^T gather table

    with tile.TileContext(nc) as tc, ExitStack() as ctx:
        const = ctx.enter_context(tc.tile_pool(name="const", bufs=1))
        emp = ctx.enter_context(tc.tile_pool(name="emp", bufs=2))
        hip = ctx.enter_context(tc.tile_pool(name="hip", bufs=2))
        wrk = ctx.enter_context(tc.tile_pool(name="wrk", bufs=2))
        psum = ctx.enter_context(tc.tile_pool(name="psum", bufs=2, space="PSUM"))

        # ---- constants -------------------------------------------------
        # Trep[b, j, i] = T[i, j]  (j-major candidate layout)
        t_ap = trans_d.ap()  # [i, j]
        tt_flat = const.tile([1, T * T], F32)
        nc.sync.dma_start(
            tt_flat[:].rearrange("p (j i) -> p j i", j=T), t_ap.transpose([1, 0]).unsqueeze(0)
        )
        # T_T[j, i] = T[i, j] on 48 partitions (rhs of the gather matmul)
        t_t = const.tile([T, T], F32)
        nc.sync.dma_start(t_t[:], t_ap.transpose([1, 0]))
        # end broadcast over batch partitions
        end_b = const.tile([B, T], F32)
        nc.sync.dma_start(end_b[:], end_d.ap().unsqueeze(0).broadcast_to([B, T]))

        # diag01[b, b'] = 1.0 iff b == b' (identity, rhs of the tag-transpose mm)
        diag_i = const.tile([B, B], I32)
        nc.gpsimd.iota(diag_i[:], pattern=[[1, B]], base=0, channel_multiplier=-1)
        diag01 = const.tile([B, B], F32)
        nc.vector.tensor_scalar(diag01[:], diag_i[:], 0, None, op0=mybir.AluOpType.is_equal)
        # iota_p[j, b] = j  (partition index, f32, on 48 partitions)
        iota_p_i = const.tile([T, B], I32)
        nc.gpsimd.iota(iota_p_i[:], pattern=[[0, B]], base=0, channel_multiplier=1)
        iota_p = const.tile([T, B], F32)
        nc.vector.tensor_copy(iota_p[:], iota_p_i[:])

        # path8[b, t, 0:8]: max_index writes full 8-wide rows; col 0 is the
        # tag. uint32 so rows double as indirect-DMA gather offsets.
        path8 = const.tile([B, S, 8], U32)
        # T^T staged to DRAM: the backtrace gathers row tag_b per partition
        nc.sync.dma_start(ttT_d.ap(), t_t[:])

        if not SPLIT_FWD:
            # Trep[b, j, i] = T[i, j] replicated across batch partitions
            trep = const.tile([B, T, T], F32)
            nc.gpsimd.partition_broadcast(
                trep[:].rearrange("p j i -> p (j i)"), tt_flat[:]
            )
            start_b = const.tile([B, T], F32)
            nc.sync.dma_start(
                start_b[:], start_d.ap().unsqueeze(0).broadcast_to([B, T])
            )

        # Trep2[g*64+b, j_lo, i] = T[i, 24g + j_lo]
        # NB: partition_broadcast ignores input free offsets on HW -- each
        # source must sit at offset 0 of its own tile.
        trep2 = start2 = None
        if SPLIT_FWD:
            tt_hi = const.tile([1, H * T], F32)
            nc.sync.dma_start(
                tt_hi[:].rearrange("p (j i) -> p j i", j=H),
                t_ap.transpose([1, 0])[H:T].unsqueeze(0),
            )
            # partition_broadcast also cannot write a partition-offset output
            # range on HW -- broadcast at base 0, then stream_shuffle up.
            trep2 = const.tile([2 * B, H, T], F32)
            nc.gpsimd.partition_broadcast(
                trep2[0:B].rearrange("p j i -> p (j i)"), tt_flat[:, 0 : H * T]
            )
            tmp_hi = const.tile([B, H, T], F32)
            nc.gpsimd.partition_broadcast(tmp_hi[:].rearrange("p j i -> p (j i)"), tt_hi[:])
            nc.vector.stream_shuffle(
                trep2[B : 2 * B].rearrange("p j i -> p (j i)"),
                tmp_hi[:].rearrange("p j i -> p (j i)"),
                mask=IDENT32,
            )
            # start2[g*64+b, j_lo] = start[24g + j_lo]
            start2 = const.tile([2 * B, H], F32)
            nc.sync.dma_start(
                start2[0:B], start_d.ap()[0:H].unsqueeze(0).broadcast_to([B, H])
            )
            nc.sync.dma_start(
                start2[B : 2 * B], start_d.ap()[H:T].unsqueeze(0).broadcast_to([B, H])
            )

        def assemble_full(s_half):
            """[128, H] half-scores -> [128, T] replicated full scores."""
            sf = wrk.tile([2 * B, T], F32, tag="sfull")
            nc.vector.tensor_copy(sf[0:B, 0:H], s_half[0:B])
            nc.vector.tensor_copy(sf[B : 2 * B, H:T], s_half[B : 2 * B])
            nc.vector.stream_shuffle(sf[0:B, H:T], s_half[B : 2 * B], mask=IDENT32)
            nc.vector.stream_shuffle(sf[B : 2 * B, 0:H], s_half[0:B], mask=IDENT32)
            return sf

        for _rep in range(reps):
            # ---- forward ---------------------------------------------------
            s_full = None
            s_last = None
            if SPLIT_FWD:
                for blk in range(nblk):
                    em_t = emp.tile([2 * B, TB, H], F32, tag="em")
                    nc.sync.dma_start(
                        em_t[0:B], em_d.ap()[:, blk * TB : (blk + 1) * TB, 0:H]
                    )
                    nc.sync.dma_start(
                        em_t[B : 2 * B], em_d.ap()[:, blk * TB : (blk + 1) * TB, H:T]
                    )
                    hist_t = hip.tile([2 * B, TB, H], F32, tag="hist")

                    for off in range(TB):
                        t = blk * TB + off
                        if t == 0:
                            nc.vector.tensor_add(
                                hist_t[:, 0, :], start2[:], em_t[:, 0, :]
                            )
                        else:
                            cand = wrk.tile([2 * B, H, T], F32, tag="cand")
                            sb = s_full[:].unsqueeze(1)
                            nc.vector.tensor_tensor(
                                cand[:],
                                sb.broadcast_to([2 * B, H, T]),
                                trep2[:],
                                op=mybir.AluOpType.add,
                            )
                            pre = wrk.tile([2 * B, H], F32, tag="pre")
                            nc.vector.tensor_reduce(
                                pre[:], cand[:], axis=mybir.AxisListType.X,
                                op=mybir.AluOpType.max,
                            )
                            nc.vector.tensor_add(
                                hist_t[:, off, :], pre[:], em_t[:, off, :]
                            )
                        s_full = assemble_full(hist_t[:, off, :])

                    nc.sync.dma_start(
                        hist_d.ap()[:, blk * TB : (blk + 1) * TB, 0:H], hist_t[0:B]
                    )
                    nc.sync.dma_start(
                        hist_d.ap()[:, blk * TB : (blk + 1) * TB, H:T],
                        hist_t[B : 2 * B],
                    )
                s_last = s_full[0:B, :]
            else:
                # unsplit: batch on 64 partitions, full j per step -- only
                # 3 instructions per step (cand, reduce, add-em), no assemble.
                hist_prev = None
                for blk in range(nblk):
                    em_t = emp.tile([B, TB, T], F32, tag="em")
                    nc.sync.dma_start(
                        em_t[:], em_d.ap()[:, blk * TB : (blk + 1) * TB, :]
                    )
                    hist_t = hip.tile([B, TB, T], F32, tag="hist")

                    for off in range(TB):
                        t = blk * TB + off
                        if t == 0:
                            nc.vector.tensor_add(
                                hist_t[:, 0, :], start_b[:], em_t[:, 0, :]
                            )
                        else:
                            s_prev = (
                                hist_t[:, off - 1, :]
                                if off > 0
                                else hist_prev[:, TB - 1, :]
                            )
                            cand = wrk.tile([B, T, T], F32, tag="cand")
                            eng_tt = nc.gpsimd if FWD_POOL else nc.vector
                            eng_tt.tensor_tensor(
                                cand[:],
                                s_prev.unsqueeze(1).broadcast_to([B, T, T]),
                                trep[:],
                                op=mybir.AluOpType.add,
                            )
                            pre = wrk.tile([B, T], F32, tag="pre")
                            nc.vector.tensor_reduce(
                                pre[:], cand[:], axis=mybir.AxisListType.X,
                                op=mybir.AluOpType.max,
                            )
                            eng_tt.tensor_tensor(
                                hist_t[:, off, :], pre[:], em_t[:, off, :],
                                op=mybir.AluOpType.add,
                            )
                    hist_prev = hist_t

                    nc.sync.dma_start(
                        hist_d.ap()[:, blk * TB : (blk + 1) * TB, :], hist_t[:]
                    )
                s_last = hist_prev[:, TB - 1, :]

            # ---- final argmax ----------------------------------------------
            fin = const.tile([B, T], F32, tag="fin")
            nc.vector.tensor_add(fin[:], s_last, end_b[:])
            m8f = const.tile([B, 8], F32, tag="m8f")
            nc.vector.max(m8f[:], fin[:])
            nc.vector.max_index(path8[:, S - 1, :], m8f[:], fin[:])

            # ---- backtrace -------------------------------------------------
            # tag_t = argmax_i(s_t[b,i] + T[i, tag_{t+1}]), recomputed exactly.
            # Chain per step: cast tag -> PE transpose (bcast lhsT) -> is_eq
            # one-hot (PSUM->SBUF) -> PE column gather -> TTR (add + fused max)
            # -> max_index (first occurrence; tie-safe via integer tags).
            TBB = 16 if BT_PSUM else TB
            nbb = S // TBB
            for rblk in (range(nbb - 1, -1, -1) if not fwd_only else []):
                hr = hip.tile([B, TBB, T], F32, tag="hist")
                nc.sync.dma_start(
                    hr[:], hist_d.ap()[:, rblk * TBB : (rblk + 1) * TBB, :]
                )
                for off in range(TBB - 1, -1, -1):
                    t = rblk * TBB + off
                    if t == S - 1:
                        continue
                    tcol = wrk.tile([B, T], F32, tag="tcol_sb")
                    nc.gpsimd.indirect_dma_start(
                        out=tcol[:],
                        out_offset=None,
                        in_=ttT_d.ap(),
                        in_offset=bass.IndirectOffsetOnAxis(
                            ap=path8[:, t + 1, 0:1], axis=0
                        ),
                    )
                    c48 = wrk.tile([B, T], F32, tag="c48")
                    nc.gpsimd.tensor_tensor(
                        c48[:], hr[:, off, :], tcol[:], op=mybir.AluOpType.add
                    )
                    m8 = wrk.tile([B, 8], F32, tag="m8")
                    nc.vector.max(m8[:], c48[:])
                    nc.vector.max_index(path8[:, t, :], m8[:], c48[:])

            # ---- emit paths -------------------------------------------------
            paths_i = const.tile([B, S], I32, tag="paths_i")
            nc.vector.tensor_copy(paths_i[:], path8[:, :, 0])
            nc.sync.dma_start(paths_d.ap()[:], paths_i[:])

    nc.compile()
    return nc


def kernel(emissions, mask, transitions, start_transitions, end_transitions):
    """Full-input entry point: shards batch over 8 cores, runs SPMD, gathers."""
    from concourse.bass_utils import run_bass_kernel_spmd

    emissions = np.ascontiguousarray(np.asarray(emissions), dtype=np.float32)
    transitions = np.ascontiguousarray(np.asarray(transitions), dtype=np.float32)
    start_transitions = np.ascontiguousarray(np.asarray(start_transitions), dtype=np.float32)
    end_transitions = np.ascontiguousarray(np.asarray(end_transitions), dtype=np.float32)

    nc = build_nc(pool_j=POOL_J)
    in_maps = []
    for c in range(N_CORES):
        sl = emissions[c * B_LOC : (c + 1) * B_LOC]
        in_maps.append(
            {
                "emissions": sl,
                "transitions": transitions,
                "start_transitions": start_transitions,
                "end_transitions": end_transitions,
            }
        )
    res = run_bass_kernel_spmd(nc, in_maps, list(range(N_CORES)))
    out = np.concatenate([r["paths"] for r in res.results], axis=0)
    return out.astype(np.int32)


# revision 27
# speedup vs baseline: 2.1910x; 1.2487x over previous
"""Trainium2 Bass kernel for CRF Viterbi decode (nn_CRFLayer).

Problem: emissions [512, 1024, 48] f32, mask [512,1024] (unused by reference),
transitions [48,48], start/end_transitions [48]. Output: best_paths [512, 1024]
int32 (Viterbi argmax decode, jax reference semantics: first-occurrence argmax).

Strategy (8 NeuronCores, pure data parallel over batch, 64 seqs/core):

The execution backend here is ~uniformly instruction-count-bound (~60-90us
per instruction regardless of operand size, measured via in-kernel reps),
so the design minimizes instructions per timestep, not modeled engine cycles.

Forward (per core): scores s_t[b, j] kept in SBUF, batch on 64 partitions,
full tag range per step -- exactly 3 instructions per timestep:
  cand[b,(j,i)] = s_{t-1}[b,i] + T[i,j]   (DVE tensor_tensor, bcast AP)
  pre[b,j]     = max_i cand               (DVE grouped tensor_reduce)
  s_t[b,j]     = pre + em_t[b,j]          (DVE tensor_add, writes hist tile)
(SPLIT_FWD=True restores the older 128-partition j-split variant: lower
modeled DVE time but 7 instructions/step -- slower on this backend.)
Scores streamed to DRAM scratch in blocks (for exact backtrace recompute).

Backtrace: tag_t = argmax_i(s_t[b,i] + T[i, tag_{t+1}]) recomputed per step,
7 instructions per timestep:
  wrep  = tag cast+broadcast u16->f32      (DVE copy)
  tagb  = PE matmul(wrep, identity)        (tag transposed to 48 partitions)
  oht   = is_eq(iota_p, tagb)              (one-hot from the *integer* tag --
                                            tie-safe, PSUM->SBUF bridge)
  tcol  = PE matmul(oht, T^T)              (per-batch column gather)
  c48   = hist_t + tcol; max8; max_index   (first-occurrence argmax)
All arithmetic is bit-exact vs the jax reference (single fp32 adds, exact max,
first-occurrence argmax), so integer paths match exactly.
"""

import os
import sys
from contextlib import ExitStack

import numpy as np

sys.path.insert(0, "/opt/trn_rl_repo")

import concourse.bass as bass  # noqa: E402
import concourse.tile as tile  # noqa: E402
from concourse import bacc, mybir  # noqa: E402

F32 = mybir.dt.float32
U16 = mybir.dt.uint16
U32 = mybir.dt.uint32
I32 = mybir.dt.int32

NUM_TAGS = 48
BATCH = 512
SEQ_LEN = 1024
N_CORES = 8
B_LOC = BATCH // N_CORES  # 64 sequences per core

NEG_INF = float(np.float32(-1e30))
USE_TTR = False
POOL_J = 0
SPLIT_FWD = False
BT_PSUM = False
FWD_POOL = True


def build_nc(
    S: int = SEQ_LEN,
    TB: int = 128,
    B: int = B_LOC,
    T: int = NUM_TAGS,
    fwd_only: bool = False,
    reps: int = 1,
    pool_j: int = 0,
    hist_out: bool = False,
):
    """Build the per-core Bass program (same program on all cores, SPMD).

    pool_j: j-columns per group-half of forward cand computed on GPSIMD.
    """
    assert S % TB == 0
    nblk = S // TB
    H = T // 2  # 24, j-half width
    IDENT32 = list(range(32))
    JD = H - pool_j  # DVE j-columns per half

    nc = bacc.Bacc("TRN2", target_bir_lowering=False, debug=False, num_devices=N_CORES)

    em_d = nc.dram_tensor("emissions", [B, S, T], F32, kind="ExternalInput")
    trans_d = nc.dram_tensor("transitions", [T, T], F32, kind="ExternalInput")
    start_d = nc.dram_tensor("start_transitions", [T], F32, kind="ExternalInput")
    end_d = nc.dram_tensor("end_transitions", [T], F32, kind="ExternalInput")
    paths_d = nc.dram_tensor("paths", [B, S], I32, kind="ExternalOutput")
    hist_d = nc.dram_tensor(
        "hist", [B, S, T], F32, kind="ExternalOutput" if hist_out else "Internal"
    )  # scratch: forward scores
    ttT_d = nc.dram_tensor("ttT", [T, T], F32, kind="Internal")  # T^T gather table

    with tile.TileContext(nc) as tc, ExitStack() as ctx:
        const = ctx.enter_context(tc.tile_pool(name="const", bufs=1))
        emp = ctx.enter_context(tc.tile_pool(name="emp", bufs=2))
        hip = ctx.enter_context(tc.tile_pool(name="hip", bufs=2))
        wrk = ctx.enter_context(tc.tile_pool(name="wrk", bufs=2))
        psum = ctx.enter_context(tc.tile_pool(name="psum", bufs=2, space="PSUM"))

        # ---- constants -------------------------------------------------
        # Trep[b, j, i] = T[i, j]  (j-major candidate layout)
        t_ap = trans_d.ap()  # [i, j]
        tt_flat = const.tile([1, T * T], F32)
        nc.sync.dma_start(
            tt_flat[:].rearrange("p (j i) -> p j i", j=T), t_ap.transpose([1, 0]).unsqueeze(0)
        )
        # T_T[j, i] = T[i, j] on 48 partitions (rhs of the gather matmul)
        t_t = const.tile([T, T], F32)
        nc.sync.dma_start(t_t[:], t_ap.transpose([1, 0]))
        # end broadcast over batch partitions
        end_b = const.tile([B, T], F32)
        nc.sync.dma_start(end_b[:], end_d.ap().unsqueeze(0).broadcast_to([B, T]))

        # diag01[b, b'] = 1.0 iff b == b' (identity, rhs of the tag-transpose mm)
        diag_i = const.tile([B, B], I32)
        nc.gpsimd.iota(diag_i[:], pattern=[[1, B]], base=0, channel_multiplier=-1)
        diag01 = const.tile([B, B], F32)
        nc.vector.tensor_scalar(diag01[:], diag_i[:], 0, None, op0=mybir.AluOpType.is_equal)
        # iota_p[j, b] = j  (partition index, f32, on 48 partitions)
        iota_p_i = const.tile([T, B], I32)
        nc.gpsimd.iota(iota_p_i[:], pattern=[[0, B]], base=0, channel_multiplier=1)
        iota_p = const.tile([T, B], F32)
        nc.vector.tensor_copy(iota_p[:], iota_p_i[:])

        # path8[b, t, 0:8]: max_index writes full 8-wide rows; col 0 is the
        # tag. uint32 so rows double as indirect-DMA gather offsets.
        path8 = const.tile([B, S, 8], U32)
        # T^T staged to DRAM: the backtrace gathers row tag_b per partition
        nc.sync.dma_start(ttT_d.ap(), t_t[:])

        if not SPLIT_FWD:
            # Trep[b, j, i] = T[i, j] replicated across batch partitions
            trep = const.tile([B, T, T], F32)
            nc.gpsimd.partition_broadcast(
                trep[:].rearrange("p j i -> p (j i)"), tt_flat[:]
            )
            start_b = const.tile([B, T], F32)
            nc.sync.dma_start(
                start_b[:], start_d.ap().unsqueeze(0).broadcast_to([B, T])
            )

        # Trep2[g*64+b, j_lo, i] = T[i, 24g + j_lo]
        # NB: partition_broadcast ignores input free offsets on HW -- each
        # source must sit at offset 0 of its own tile.
        trep2 = start2 = None
        if SPLIT_FWD:
            tt_hi = const.tile([1, H * T], F32)
            nc.sync.dma_start(
                tt_hi[:].rearrange("p (j i) -> p j i", j=H),
                t_ap.transpose([1, 0])[H:T].unsqueeze(0),
            )
            # partition_broadcast also cannot write a partition-offset output
            # range on HW -- broadcast at base 0, then stream_shuffle up.
            trep2 = const.tile([2 * B, H, T], F32)
            nc.gpsimd.partition_broadcast(
                trep2[0:B].rearrange("p j i -> p (j i)"), tt_flat[:, 0 : H * T]
            )
            tmp_hi = const.tile([B, H, T], F32)
            nc.gpsimd.partition_broadcast(tmp_hi[:].rearrange("p j i -> p (j i)"), tt_hi[:])
            nc.vector.stream_shuffle(
                trep2[B : 2 * B].rearrange("p j i -> p (j i)"),
                tmp_hi[:].rearrange("p j i -> p (j i)"),
                mask=IDENT32,
            )
            # start2[g*64+b, j_lo] = start[24g + j_lo]
            start2 = const.tile([2 * B, H], F32)
            nc.sync.dma_start(
                start2[0:B], start_d.ap()[0:H].unsqueeze(0).broadcast_to([B, H])
            )
            nc.sync.dma_start(
                start2[B : 2 * B], start_d.ap()[H:T].unsqueeze(0).broadcast_to([B, H])
            )

        def assemble_full(s_half):
            """[128, H] half-scores -> [128, T] replicated full scores."""
            sf = wrk.tile([2 * B, T], F32, tag="sfull")
            nc.vector.tensor_copy(sf[0:B, 0:H], s_half[0:B])
            nc.vector.tensor_copy(sf[B : 2 * B, H:T], s_half[B : 2 * B])
            nc.vector.stream_shuffle(sf[0:B, H:T], s_half[B : 2 * B], mask=IDENT32)
            nc.vector.stream_shuffle(sf[B : 2 * B, 0:H], s_half[0:B], mask=IDENT32)
            return sf

        for _rep in range(reps):
            # ---- forward ---------------------------------------------------
            s_full = None
            s_last = None
            if SPLIT_FWD:
                for blk in range(nblk):
                    em_t = emp.tile([2 * B, TB, H], F32, tag="em")
                    nc.sync.dma_start(
                        em_t[0:B], em_d.ap()[:, blk * TB : (blk + 1) * TB, 0:H]
                    )
                    nc.sync.dma_start(
                        em_t[B : 2 * B], em_d.ap()[:, blk * TB : (blk + 1) * TB, H:T]
                    )
                    hist_t = hip.tile([2 * B, TB, H], F32, tag="hist")

                    for off in range(TB):
                        t = blk * TB + off
                        if t == 0:
                            nc.vector.tensor_add(
                                hist_t[:, 0, :], start2[:], em_t[:, 0, :]
                            )
                        else:
                            cand = wrk.tile([2 * B, H, T], F32, tag="cand")
                            sb = s_full[:].unsqueeze(1)
                            nc.vector.tensor_tensor(
                                cand[:],
                                sb.broadcast_to([2 * B, H, T]),
                                trep2[:],
                                op=mybir.AluOpType.add,
                            )
                            pre = wrk.tile([2 * B, H], F32, tag="pre")
                            nc.vector.tensor_reduce(
                                pre[:], cand[:], axis=mybir.AxisListType.X,
                                op=mybir.AluOpType.max,
                            )
                            nc.vector.tensor_add(
                                hist_t[:, off, :], pre[:], em_t[:, off, :]
                            )
                        s_full = assemble_full(hist_t[:, off, :])

                    nc.sync.dma_start(
                        hist_d.ap()[:, blk * TB : (blk + 1) * TB, 0:H], hist_t[0:B]
                    )
                    nc.sync.dma_start(
                        hist_d.ap()[:, blk * TB : (blk + 1) * TB, H:T],
                        hist_t[B : 2 * B],
                    )
                s_last = s_full[0:B, :]
            else:
                # unsplit: batch on 64 partitions, full j per step -- only
                # 3 instructions per step (cand, reduce, add-em), no assemble.
                hist_prev = None
                for blk in range(nblk):
                    em_t = emp.tile([B, TB, T], F32, tag="em")
                    nc.sync.dma_start(
                        em_t[:], em_d.ap()[:, blk * TB : (blk + 1) * TB, :]
                    )
                    hist_t = hip.tile([B, TB, T], F32, tag="hist")

                    for off in range(TB):
                        t = blk * TB + off
                        if t == 0:
                            nc.vector.tensor_add(
                                hist_t[:, 0, :], start_b[:], em_t[:, 0, :]
                            )
                        else:
                            s_prev = (
                                hist_t[:, off - 1, :]
                                if off > 0
                                else hist_prev[:, TB - 1, :]
                            )
                            cand = wrk.tile([B, T, T], F32, tag="cand")
                            eng_tt = nc.gpsimd if FWD_POOL else nc.vector
                            eng_tt.tensor_tensor(
                                cand[:],
                                s_prev.unsqueeze(1).broadcast_to([B, T, T]),
                                trep[:],
                                op=mybir.AluOpType.add,
                            )
                            pre = wrk.tile([B, T], F32, tag="pre")
                            nc.vector.tensor_reduce(
                                pre[:], cand[:], axis=mybir.AxisListType.X,
                                op=mybir.AluOpType.max,
                            )
                            eng_tt.tensor_tensor(
                                hist_t[:, off, :], pre[:], em_t[:, off, :],
                                op=mybir.AluOpType.add,
                            )
                    hist_prev = hist_t

                    nc.sync.dma_start(
                        hist_d.ap()[:, blk * TB : (blk + 1) * TB, :], hist_t[:]
                    )
                s_last = hist_prev[:, TB - 1, :]

            # ---- final argmax ----------------------------------------------
            fin = const.tile([B, T], F32, tag="fin")
            nc.vector.tensor_add(fin[:], s_last, end_b[:])
            m8f = const.tile([B, 8], F32, tag="m8f")
            nc.vector.max(m8f[:], fin[:])
            nc.vector.max_index(path8[:, S - 1, :], m8f[:], fin[:])

            # ---- backtrace -------------------------------------------------
            # tag_t = argmax_i(s_t[b,i] + T[i, tag_{t+1}]), recomputed exactly.
            # Chain per step: cast tag -> PE transpose (bcast lhsT) -> is_eq
            # one-hot (PSUM->SBUF) -> PE column gather -> TTR (add + fused max)
            # -> max_index (first occurrence; tie-safe via integer tags).
            TBB = 16 if BT_PSUM else TB
            nbb = S // TBB
            for rblk in (range(nbb - 1, -1, -1) if not fwd_only else []):
                hr = hip.tile([B, TBB, T], F32, tag="hist")
                nc.sync.dma_start(
                    hr[:], hist_d.ap()[:, rblk * TBB : (rblk + 1) * TBB, :]
                )
                for off in range(TBB - 1, -1, -1):
                    t = rblk * TBB + off
                    if t == S - 1:
                        continue
                    # gather T^T[tag_b, :] and CCE-accumulate it straight
                    # onto the history row: c48 = hist_t + T[:, tag] in one
                    # DMA instruction (hr[:, off] is consumed exactly once).
                    nc.gpsimd.indirect_dma_start(
                        out=hr[:, off, :],
                        out_offset=None,
                        in_=ttT_d.ap(),
                        in_offset=bass.IndirectOffsetOnAxis(
                            ap=path8[:, t + 1, 0:1], axis=0
                        ),
                        compute_op=mybir.AluOpType.add,
                    )
                    m8 = wrk.tile([B, 8], F32, tag="m8")
                    nc.vector.max(m8[:], hr[:, off, :])
                    nc.vector.max_index(path8[:, t, :], m8[:], hr[:, off, :])

            # ---- emit paths -------------------------------------------------
            paths_i = const.tile([B, S], I32, tag="paths_i")
            nc.vector.tensor_copy(paths_i[:], path8[:, :, 0])
            nc.sync.dma_start(paths_d.ap()[:], paths_i[:])

    nc.compile()
    return nc


def kernel(emissions, mask, transitions, start_transitions, end_transitions):
    """Full-input entry point: shards batch over 8 cores, runs SPMD, gathers."""
    from concourse.bass_utils import run_bass_kernel_spmd

    emissions = np.ascontiguousarray(np.asarray(emissions), dtype=np.float32)
    transitions = np.ascontiguousarray(np.asarray(transitions), dtype=np.float32)
    start_transitions = np.ascontiguousarray(np.asarray(start_transitions), dtype=np.float32)
    end_transitions = np.ascontiguousarray(np.asarray(end_transitions), dtype=np.float32)

    nc = build_nc(pool_j=POOL_J)
    in_maps = []
    for c in range(N_CORES):
        sl = emissions[c * B_LOC : (c + 1) * B_LOC]
        in_maps.append(
            {
                "emissions": sl,
                "transitions": transitions,
                "start_transitions": start_transitions,
                "end_transitions": end_transitions,
            }
        )
    res = run_bass_kernel_spmd(nc, in_maps, list(range(N_CORES)))
    out = np.concatenate([r["paths"] for r in res.results], axis=0)
    return out.astype(np.int32)
